# revision 5
# baseline (speedup 1.0000x reference)
"""AttentionPooling (segment softmax + weighted segment sum) on 8 trn2 cores.

Strategy: shard whole segments across cores (sorted batch -> contiguous node
ranges), pad each core's slice to a common node count, run one SPMD Bass/Tile
program.  Per 2048-node chunk: SWDGE cast-load x (fp32->bf16, node-partitioned),
xbar SB->SB transpose to channel-partitioned, PE matmuls for the MLP score,
exp on ACT, onehot(segment)*e stationary operand for the weighted-sum matmul
accumulating (64,256) + denominators in PSUM.  Softmax max-subtraction is
skipped: |s| <= ||W2||_1 + |b2| ~ 28, exp stays in fp32 range.
"""

from contextlib import ExitStack

import numpy as np
import ml_dtypes

import concourse.bass as bass
import concourse.bacc as bacc
import concourse.tile as tile
from concourse import mybir
from concourse.bass_utils import run_bass_kernel_spmd

N_CORES = 8
NUM_GRAPHS = 512
SEGS_PER_CORE = NUM_GRAPHS // N_CORES  # 64
D = 256          # in channels
H = 128          # hidden
P = 128          # partitions
TILE_N = 128     # nodes per weight tile
CHUNK_T = 16     # tiles per chunk
CHUNK_N = TILE_N * CHUNK_T  # 2048 nodes per chunk

_BF16 = mybir.dt.bfloat16
_F32 = mybir.dt.float32
_I32 = mybir.dt.int32


def _build_program(n_chunks: int, b2_val: float):
    nc = bacc.Bacc()
    nmax = n_chunks * CHUNK_N
    nt = nmax // TILE_N

    x_d = nc.declare_dram_parameter("x", [nmax, D], _F32, isOutput=False)
    bt_d = nc.declare_dram_parameter("batch_t", [P, nt + SEGS_PER_CORE], _I32, isOutput=False)
    w1_d = nc.declare_dram_parameter("w1", [D, H], _BF16, isOutput=False)
    w2_d = nc.declare_dram_parameter("w2", [H, 2], _BF16, isOutput=False)
    b1_d = nc.declare_dram_parameter("b1", [H, 1], _F32, isOutput=False)
    out_d = nc.declare_dram_parameter("out_g", [SEGS_PER_CORE, D], _F32, isOutput=True)

    # chunked view of x: chunk c -> (p=node%128, t=tile-in-chunk, ch)
    x_ap = x_d[:].rearrange("(c t p) ch -> c p t ch", p=P, t=CHUNK_T)

    with tile.TileContext(nc) as tc, ExitStack() as ctx:
        const_pool = ctx.enter_context(tc.tile_pool(name="consts", bufs=1))
        xbf_pool = ctx.enter_context(tc.tile_pool(name="xbf", bufs=3))
        xt_pool = ctx.enter_context(tc.tile_pool(name="xt", bufs=2))
        h_pool = ctx.enter_context(tc.tile_pool(name="h", bufs=2))
        we_pool = ctx.enter_context(tc.tile_pool(name="we", bufs=2))
        ecol_pool = ctx.enter_context(tc.tile_pool(name="ecol", bufs=2))
        fin_pool = ctx.enter_context(tc.tile_pool(name="fin", bufs=1))
        psum_h = ctx.enter_context(
            tc.tile_pool(name="psum_h", bufs=2, space=bass.MemorySpace.PSUM))
        psum_s = ctx.enter_context(
            tc.tile_pool(name="psum_s", bufs=2, space=bass.MemorySpace.PSUM))
        psum_acc = ctx.enter_context(
            tc.tile_pool(name="psum_acc", bufs=1, space=bass.MemorySpace.PSUM))

        # ---- constants / weights ----
        w1_sb = const_pool.tile([P, 2, H], _BF16, tag="w1")   # [:, 0, :]=ch 0-127
        nc.sync.dma_start(w1_sb[:, 0, :], w1_d[0:128, :])
        nc.sync.dma_start(w1_sb[:, 1, :], w1_d[128:256, :])
        w2o_sb = const_pool.tile([P, 2], _BF16, tag="w2")
        nc.sync.dma_start(w2o_sb[:], w2_d[:])
        w2_sb = w2o_sb[:, 0:1]
        ones_sb = w2o_sb[:, 1:2]
        b1_sb = const_pool.tile([P, 1], _F32, tag="b1")
        nc.sync.dma_start(b1_sb[:], b1_d[:])
        bt_sb = const_pool.tile([P, nt + SEGS_PER_CORE], _I32, tag="bt")
        nc.sync.dma_start(bt_sb[:], bt_d[:])
        iota_sb = bt_sb[:, nt:nt + SEGS_PER_CORE]

        acc_ps = psum_acc.tile([SEGS_PER_CORE, D], _F32, tag="acc")
        den_ps = psum_acc.tile([SEGS_PER_CORE, 1], _F32, tag="den")

        for c in range(n_chunks):
            # 1) load + cast x chunk (node-partitioned)
            x_bf = xbf_pool.tile([P, CHUNK_T, D], _BF16, tag="xbf")
            nc.gpsimd.dma_start(x_bf[:], x_ap[c])

            # 2) xbar transpose -> channel-partitioned halves (128, CHUNK_N)
            xt_lo = xt_pool.tile([P, CHUNK_N], _BF16, tag="xtlo")
            xt_hi = xt_pool.tile([P, CHUNK_N], _BF16, tag="xthi")
            for t in range(CHUNK_T):
                nc.sync.dma_start_transpose(
                    xt_lo[:, t * TILE_N:(t + 1) * TILE_N], x_bf[:, t, 0:128])
                nc.sync.dma_start_transpose(
                    xt_hi[:, t * TILE_N:(t + 1) * TILE_N], x_bf[:, t, 128:256])

            # 3) h = tanh(x @ W1 + b1), hidden-partitioned, bf16
            h_bf = h_pool.tile([P, CHUNK_N], _BF16, tag="h")
            for s in range(CHUNK_N // 512):
                ph = psum_h.tile([P, 512], _F32, tag="ph")
                sl = slice(s * 512, (s + 1) * 512)
                nc.tensor.matmul(ph[:], w1_sb[:, 0, :], xt_lo[:, sl],
                                 start=True, stop=False)
                nc.tensor.matmul(ph[:], w1_sb[:, 1, :], xt_hi[:, sl],
                                 start=False, stop=True)
                nc.scalar.activation(h_bf[:, sl], ph[:],
                                     mybir.ActivationFunctionType.Tanh,
                                     bias=b1_sb[:])

            # 4) per-tile score columns: s_col[p, t] = h_tile.T @ W2
            ps_s = psum_s.tile([P, CHUNK_T], _F32, tag="ps_s")
            for t in range(CHUNK_T):
                nc.tensor.matmul(ps_s[:, t:t + 1],
                                 h_bf[:, t * TILE_N:(t + 1) * TILE_N],
                                 w2_sb, start=True, stop=True)

            # 5) e = exp(s + b2)  (node-partitioned, fp32)
            e_col = ecol_pool.tile([P, CHUNK_T], _F32, tag="ecol")
            nc.scalar.activation(e_col[:], ps_s[:],
                                 mybir.ActivationFunctionType.Exp,
                                 bias=float(b2_val))

            # 6) we[p, t, g] = (batch_t == g) * e   (bf16)
            cmp = we_pool.tile([P, CHUNK_T, SEGS_PER_CORE], _BF16, tag="cmp")
            bt_c = bt_sb[:, c * CHUNK_T:(c + 1) * CHUNK_T]
            nc.vector.tensor_tensor(
                cmp[:],
                bt_c.unsqueeze(2).broadcast_to([P, CHUNK_T, SEGS_PER_CORE]),
                iota_sb.unsqueeze(1).broadcast_to([P, CHUNK_T, SEGS_PER_CORE]),
                mybir.AluOpType.is_equal)
            we = we_pool.tile([P, CHUNK_T, SEGS_PER_CORE], _BF16, tag="we")
            nc.vector.memset(we[:], 0.0)
            nc.vector.tensor_tensor(
                we[:], cmp[:],
                e_col[:].unsqueeze(2).broadcast_to([P, CHUNK_T, SEGS_PER_CORE]),
                mybir.AluOpType.mult)

            # 7) weighted segment sums + denominators, accumulated in PSUM
            first = c == 0
            last = c == n_chunks - 1
            for t in range(CHUNK_T):
                nc.tensor.matmul(acc_ps[:], we[:, t, :], x_bf[:, t, :],
                                 start=(first and t == 0),
                                 stop=(last and t == CHUNK_T - 1),
                                 skip_group_check=True)
                nc.tensor.matmul(den_ps[:], we[:, t, :], ones_sb,
                                 start=(first and t == 0),
                                 stop=(last and t == CHUNK_T - 1),
                                 skip_group_check=True)

        # ---- epilogue: out = acc / den ----
        den_sb = fin_pool.tile([SEGS_PER_CORE, 1], _F32, tag="den_sb")
        nc.vector.tensor_scalar_add(den_sb[:], den_ps[:], 1e-30)
        rec_sb = fin_pool.tile([SEGS_PER_CORE, 1], _F32, tag="rec_sb")
        nc.vector.reciprocal(rec_sb[:], den_sb[:])
        out_sb = fin_pool.tile([SEGS_PER_CORE, D], _F32, tag="out_sb")
        nc.vector.tensor_scalar_mul(out_sb[:], acc_ps[:], rec_sb[:])
        nc.sync.dma_start(out_d[:], out_sb[:])

    return nc


def _prepare_inputs(x, W1, b1, W2, b2, batch):
    n = x.shape[0]
    batch = np.asarray(batch).astype(np.int64)
    # core k owns segments [64k, 64(k+1)); sorted batch -> contiguous ranges
    bounds = np.searchsorted(batch, np.arange(0, NUM_GRAPHS + 1, SEGS_PER_CORE))
    counts = np.diff(bounds)
    nmax = int(np.max(counts))
    n_chunks = max(1, (nmax + CHUNK_N - 1) // CHUNK_N)
    nmax_pad = n_chunks * CHUNK_N

    w1_bf = np.asarray(W1, np.float32).astype(ml_dtypes.bfloat16)
    w2_bf = np.concatenate([np.asarray(W2, np.float32).reshape(H, 1),
                            np.ones((H, 1), np.float32)], 1).astype(ml_dtypes.bfloat16)
    b1_col = np.asarray(b1, np.float32).reshape(H, 1)

    in_maps = []
    for k in range(N_CORES):
        lo, hi = int(bounds[k]), int(bounds[k + 1])
        cnt = hi - lo
        x_pad = np.zeros((nmax_pad, D), np.float32)
        x_pad[:cnt] = x[lo:hi]
        bt = np.full((nmax_pad,), -1, np.int32)
        bt[:cnt] = batch[lo:hi] - k * SEGS_PER_CORE
        bt_t = bt.reshape(nmax_pad // P, P).T  # (128, nt)
        iota_cols = np.tile(np.arange(SEGS_PER_CORE, dtype=np.int32), (P, 1))
        bt_t = np.concatenate([bt_t, iota_cols], axis=1).copy()
        in_maps.append({
            "x": x_pad,
            "batch_t": bt_t,
            "w1": w1_bf,
            "w2": w2_bf,
            "b1": b1_col,
        })
    return in_maps, n_chunks


def run(x, W1, b1, W2, b2, batch, trace=False, trace_kwargs=None):
    in_maps, n_chunks = _prepare_inputs(x, W1, b1, W2, b2, batch)
    nc = _build_program(n_chunks, float(np.asarray(b2).reshape(-1)[0]))
    nc.finalize()
    res = run_bass_kernel_spmd(nc, in_maps, list(range(N_CORES)),
                               trace=trace, **(trace_kwargs or {}))
    out = np.concatenate([np.asarray(res.results[k]["out_g"], np.float32)
                          for k in range(N_CORES)], axis=0)
    return out, res


def kernel(x, W1, b1, W2, b2, batch):
    out, _ = run(x, W1, b1, W2, b2, batch)
    return out


# revision 6
# speedup vs baseline: 9.9888x; 9.9888x over previous
"""AttentionPooling (segment softmax + weighted segment sum) on 8 trn2 cores.

Strategy: shard whole segments across cores (sorted batch -> contiguous node
ranges), pad each core's slice to a common node count, run one SPMD Bass/Tile
program.  Per 2048-node chunk: SWDGE cast-load x (fp32->bf16, node-partitioned),
xbar SB->SB transpose to channel-partitioned, PE matmuls for the MLP score,
exp on ACT, onehot(segment)*e stationary operand for the weighted-sum matmul
accumulating (64,256) + denominators in PSUM.  Softmax max-subtraction is
skipped: |s| <= ||W2||_1 + |b2| ~ 28, exp stays in fp32 range.
"""

from contextlib import ExitStack

import numpy as np
import ml_dtypes

import concourse.bass as bass
import concourse.bacc as bacc
import concourse.tile as tile
from concourse import mybir
from concourse.bass_utils import run_bass_kernel_spmd

N_CORES = 8
NUM_GRAPHS = 512
SEGS_PER_CORE = NUM_GRAPHS // N_CORES  # 64
D = 256          # in channels
H = 128          # hidden
P = 128          # partitions
TILE_N = 128     # nodes per weight tile
CHUNK_T = 16     # tiles per chunk
CHUNK_N = TILE_N * CHUNK_T  # 2048 nodes per chunk

_BF16 = mybir.dt.bfloat16
_F32 = mybir.dt.float32
_I32 = mybir.dt.int32


def _build_program(n_chunks: int, b2_val: float, reps: int = 1):
    nc = bacc.Bacc()
    nmax = n_chunks * CHUNK_N
    nt = nmax // TILE_N

    x_d = nc.declare_dram_parameter("x", [nmax, D], _F32, isOutput=False)
    bt_d = nc.declare_dram_parameter("batch_t", [P, nt + SEGS_PER_CORE], _I32, isOutput=False)
    w1_d = nc.declare_dram_parameter("w1", [D, H], _BF16, isOutput=False)
    w2_d = nc.declare_dram_parameter("w2", [H, 2], _BF16, isOutput=False)
    b1_d = nc.declare_dram_parameter("b1", [H, 1], _F32, isOutput=False)
    out_d = nc.declare_dram_parameter("out_g", [SEGS_PER_CORE, D], _F32, isOutput=True)

    # chunked view of x: chunk c -> (p=node%128, t=tile-in-chunk, ch)
    x_ap = x_d[:].rearrange("(c t p) ch -> c p t ch", p=P, t=CHUNK_T)

    with tile.TileContext(nc) as tc, ExitStack() as ctx:
        const_pool = ctx.enter_context(tc.tile_pool(name="consts", bufs=1))
        xbf_pool = ctx.enter_context(tc.tile_pool(name="xbf", bufs=3))
        xt_pool = ctx.enter_context(tc.tile_pool(name="xt", bufs=2))
        h_pool = ctx.enter_context(tc.tile_pool(name="h", bufs=2))
        we_pool = ctx.enter_context(tc.tile_pool(name="we", bufs=2))
        ecol_pool = ctx.enter_context(tc.tile_pool(name="ecol", bufs=2))
        fin_pool = ctx.enter_context(tc.tile_pool(name="fin", bufs=1))
        psum_h = ctx.enter_context(
            tc.tile_pool(name="psum_h", bufs=2, space=bass.MemorySpace.PSUM))
        psum_s = ctx.enter_context(
            tc.tile_pool(name="psum_s", bufs=2, space=bass.MemorySpace.PSUM))
        psum_acc = ctx.enter_context(
            tc.tile_pool(name="psum_acc", bufs=1, space=bass.MemorySpace.PSUM))

        # ---- constants / weights ----
        w1_sb = const_pool.tile([P, 2, H], _BF16, tag="w1")   # [:, 0, :]=ch 0-127
        nc.sync.dma_start(w1_sb[:, 0, :], w1_d[0:128, :])
        nc.sync.dma_start(w1_sb[:, 1, :], w1_d[128:256, :])
        w2o_sb = const_pool.tile([P, 2], _BF16, tag="w2")
        nc.sync.dma_start(w2o_sb[:], w2_d[:])
        w2_sb = w2o_sb[:, 0:1]
        ones_sb = w2o_sb[:, 1:2]
        b1_sb = const_pool.tile([P, 1], _F32, tag="b1")
        nc.sync.dma_start(b1_sb[:], b1_d[:])
        bt_sb = const_pool.tile([P, nt + SEGS_PER_CORE], _I32, tag="bt")
        nc.sync.dma_start(bt_sb[:], bt_d[:])
        iota_sb = bt_sb[:, nt:nt + SEGS_PER_CORE]

        acc_ps = psum_acc.tile([SEGS_PER_CORE, D], _F32, tag="acc")
        den_ps = psum_acc.tile([SEGS_PER_CORE, 1], _F32, tag="den")

        rep_ctx = tc.For_i(0, reps, 1) if reps > 1 else None
        if rep_ctx is not None:
            rep_ctx.__enter__()
        for c in range(n_chunks):
            # 1) load + cast x chunk (node-partitioned)
            x_bf = xbf_pool.tile([P, CHUNK_T, D], _BF16, tag="xbf")
            nc.gpsimd.dma_start(x_bf[:], x_ap[c])

            # 2) xbar transpose -> channel-partitioned halves (128, CHUNK_N)
            xt_lo = xt_pool.tile([P, CHUNK_N], _BF16, tag="xtlo")
            xt_hi = xt_pool.tile([P, CHUNK_N], _BF16, tag="xthi")
            for t in range(CHUNK_T):
                nc.sync.dma_start_transpose(
                    xt_lo[:, t * TILE_N:(t + 1) * TILE_N], x_bf[:, t, 0:128])
                nc.sync.dma_start_transpose(
                    xt_hi[:, t * TILE_N:(t + 1) * TILE_N], x_bf[:, t, 128:256])

            # 3) h = tanh(x @ W1 + b1), hidden-partitioned, bf16
            h_bf = h_pool.tile([P, CHUNK_N], _BF16, tag="h")
            for s in range(CHUNK_N // 512):
                ph = psum_h.tile([P, 512], _F32, tag="ph")
                sl = slice(s * 512, (s + 1) * 512)
                nc.tensor.matmul(ph[:], w1_sb[:, 0, :], xt_lo[:, sl],
                                 start=True, stop=False)
                nc.tensor.matmul(ph[:], w1_sb[:, 1, :], xt_hi[:, sl],
                                 start=False, stop=True)
                nc.scalar.activation(h_bf[:, sl], ph[:],
                                     mybir.ActivationFunctionType.Tanh,
                                     bias=b1_sb[:])

            # 4) per-tile score columns: s_col[p, t] = h_tile.T @ W2
            ps_s = psum_s.tile([P, CHUNK_T], _F32, tag="ps_s")
            for t in range(CHUNK_T):
                nc.tensor.matmul(ps_s[:, t:t + 1],
                                 h_bf[:, t * TILE_N:(t + 1) * TILE_N],
                                 w2_sb, start=True, stop=True)

            # 5) e = exp(s + b2)  (node-partitioned, fp32)
            e_col = ecol_pool.tile([P, CHUNK_T], _F32, tag="ecol")
            nc.scalar.activation(e_col[:], ps_s[:],
                                 mybir.ActivationFunctionType.Exp,
                                 bias=float(b2_val))

            # 6) we[p, t, g] = (batch_t == g) * e   (bf16)
            cmp = we_pool.tile([P, CHUNK_T, SEGS_PER_CORE], _BF16, tag="cmp")
            bt_c = bt_sb[:, c * CHUNK_T:(c + 1) * CHUNK_T]
            nc.vector.tensor_tensor(
                cmp[:],
                bt_c.unsqueeze(2).broadcast_to([P, CHUNK_T, SEGS_PER_CORE]),
                iota_sb.unsqueeze(1).broadcast_to([P, CHUNK_T, SEGS_PER_CORE]),
                mybir.AluOpType.is_equal)
            we = we_pool.tile([P, CHUNK_T, SEGS_PER_CORE], _BF16, tag="we")
            nc.vector.memset(we[:], 0.0)
            nc.vector.tensor_tensor(
                we[:], cmp[:],
                e_col[:].unsqueeze(2).broadcast_to([P, CHUNK_T, SEGS_PER_CORE]),
                mybir.AluOpType.mult)

            # 7) weighted segment sums + denominators, accumulated in PSUM
            first = c == 0
            last = c == n_chunks - 1
            for t in range(CHUNK_T):
                nc.tensor.matmul(acc_ps[:], we[:, t, :], x_bf[:, t, :],
                                 start=(first and t == 0),
                                 stop=(last and t == CHUNK_T - 1),
                                 skip_group_check=True)
                nc.tensor.matmul(den_ps[:], we[:, t, :], ones_sb,
                                 start=(first and t == 0),
                                 stop=(last and t == CHUNK_T - 1),
                                 skip_group_check=True)

        if rep_ctx is not None:
            rep_ctx.__exit__(None, None, None)

        # ---- epilogue: out = acc / den ----
        den_sb = fin_pool.tile([SEGS_PER_CORE, 1], _F32, tag="den_sb")
        nc.vector.tensor_scalar_add(den_sb[:], den_ps[:], 1e-30)
        rec_sb = fin_pool.tile([SEGS_PER_CORE, 1], _F32, tag="rec_sb")
        nc.vector.reciprocal(rec_sb[:], den_sb[:])
        out_sb = fin_pool.tile([SEGS_PER_CORE, D], _F32, tag="out_sb")
        nc.vector.tensor_scalar_mul(out_sb[:], acc_ps[:], rec_sb[:])
        nc.sync.dma_start(out_d[:], out_sb[:])

    return nc


def _prepare_inputs(x, W1, b1, W2, b2, batch):
    n = x.shape[0]
    batch = np.asarray(batch).astype(np.int64)
    # core k owns segments [64k, 64(k+1)); sorted batch -> contiguous ranges
    bounds = np.searchsorted(batch, np.arange(0, NUM_GRAPHS + 1, SEGS_PER_CORE))
    counts = np.diff(bounds)
    nmax = int(np.max(counts))
    n_chunks = max(1, (nmax + CHUNK_N - 1) // CHUNK_N)
    nmax_pad = n_chunks * CHUNK_N

    w1_bf = np.asarray(W1, np.float32).astype(ml_dtypes.bfloat16)
    w2_bf = np.concatenate([np.asarray(W2, np.float32).reshape(H, 1),
                            np.ones((H, 1), np.float32)], 1).astype(ml_dtypes.bfloat16)
    b1_col = np.asarray(b1, np.float32).reshape(H, 1)

    in_maps = []
    for k in range(N_CORES):
        lo, hi = int(bounds[k]), int(bounds[k + 1])
        cnt = hi - lo
        x_pad = np.zeros((nmax_pad, D), np.float32)
        x_pad[:cnt] = x[lo:hi]
        bt = np.full((nmax_pad,), -1, np.int32)
        bt[:cnt] = batch[lo:hi] - k * SEGS_PER_CORE
        bt_t = bt.reshape(nmax_pad // P, P).T  # (128, nt)
        iota_cols = np.tile(np.arange(SEGS_PER_CORE, dtype=np.int32), (P, 1))
        bt_t = np.concatenate([bt_t, iota_cols], axis=1).copy()
        in_maps.append({
            "x": x_pad,
            "batch_t": bt_t,
            "w1": w1_bf,
            "w2": w2_bf,
            "b1": b1_col,
        })
    return in_maps, n_chunks


def run(x, W1, b1, W2, b2, batch, trace=False, trace_kwargs=None):
    in_maps, n_chunks = _prepare_inputs(x, W1, b1, W2, b2, batch)
    nc = _build_program(n_chunks, float(np.asarray(b2).reshape(-1)[0]))
    nc.finalize()
    res = run_bass_kernel_spmd(nc, in_maps, list(range(N_CORES)),
                               trace=trace, **(trace_kwargs or {}))
    out = np.concatenate([np.asarray(res.results[k]["out_g"], np.float32)
                          for k in range(N_CORES)], axis=0)
    return out, res


def kernel(x, W1, b1, W2, b2, batch):
    out, _ = run(x, W1, b1, W2, b2, batch)
    return out


# revision 7
# speedup vs baseline: 25.2038x; 2.5232x over previous
"""AttentionPooling (segment softmax + weighted segment sum) on 8 trn2 cores.

Strategy: shard whole segments across cores (sorted batch -> contiguous node
ranges), pad each core's slice to a common node count, run one SPMD Bass/Tile
program.  Per 2048-node chunk: SWDGE cast-load x (fp32->bf16, node-partitioned),
xbar SB->SB transpose to channel-partitioned, PE matmuls for the MLP score,
exp on ACT, onehot(segment)*e stationary operand for the weighted-sum matmul
accumulating (64,256) + denominators in PSUM.  Softmax max-subtraction is
skipped: |s| <= ||W2||_1 + |b2| ~ 28, exp stays in fp32 range.
"""

from contextlib import ExitStack

import numpy as np
import ml_dtypes

import concourse.bass as bass
import concourse.bacc as bacc
import concourse.tile as tile
from concourse import mybir
from concourse.bass_utils import run_bass_kernel_spmd

N_CORES = 8
NUM_GRAPHS = 512
SEGS_PER_CORE = NUM_GRAPHS // N_CORES  # 64
D = 256          # in channels
H = 128          # hidden
P = 128          # partitions
TILE_N = 128     # nodes per weight tile
CHUNK_T = 16     # tiles per chunk
CHUNK_N = TILE_N * CHUNK_T  # 2048 nodes per chunk

_BF16 = mybir.dt.bfloat16
_F32 = mybir.dt.float32
_I32 = mybir.dt.int32


def _build_program(n_chunks: int, b2_val: float, reps: int = 1):
    nc = bacc.Bacc()
    nmax = n_chunks * CHUNK_N
    nt = nmax // TILE_N

    x_d = nc.declare_dram_parameter("x", [nmax, D], _F32, isOutput=False)
    bt_d = nc.declare_dram_parameter("batch_t", [P, nt + SEGS_PER_CORE], _I32, isOutput=False)
    w1_d = nc.declare_dram_parameter("w1", [D, H], _BF16, isOutput=False)
    w2_d = nc.declare_dram_parameter("w2", [H, 2], _BF16, isOutput=False)
    b1_d = nc.declare_dram_parameter("b1", [H, 1], _F32, isOutput=False)
    out_d = nc.declare_dram_parameter("out_g", [SEGS_PER_CORE, D], _F32, isOutput=True)

    # chunked view of x: chunk c -> (p=node%128, t=tile-in-chunk, ch)
    x_ap = x_d[:].rearrange("(c t p) ch -> c p t ch", p=P, t=CHUNK_T)

    with tile.TileContext(nc) as tc, ExitStack() as ctx:
        const_pool = ctx.enter_context(tc.tile_pool(name="consts", bufs=1))
        xbf_pool = ctx.enter_context(tc.tile_pool(name="xbf", bufs=3))
        xt_pool = ctx.enter_context(tc.tile_pool(name="xt", bufs=3))
        h_pool = ctx.enter_context(tc.tile_pool(name="h", bufs=2))
        we_pool = ctx.enter_context(tc.tile_pool(name="we", bufs=2))
        ecol_pool = ctx.enter_context(tc.tile_pool(name="ecol", bufs=2))
        fin_pool = ctx.enter_context(tc.tile_pool(name="fin", bufs=1))
        xd_pool = ctx.enter_context(
            tc.tile_pool(name="xbounce", bufs=4, space=bass.MemorySpace.DRAM))
        psum_h = ctx.enter_context(
            tc.tile_pool(name="psum_h", bufs=2, space=bass.MemorySpace.PSUM))
        psum_s = ctx.enter_context(
            tc.tile_pool(name="psum_s", bufs=2, space=bass.MemorySpace.PSUM))
        psum_acc = ctx.enter_context(
            tc.tile_pool(name="psum_acc", bufs=1, space=bass.MemorySpace.PSUM))

        # ---- constants / weights ----
        w1_sb = const_pool.tile([P, 2, H], _BF16, tag="w1")   # [:, 0, :]=ch 0-127
        nc.sync.dma_start(w1_sb[:, 0, :], w1_d[0:128, :])
        nc.sync.dma_start(w1_sb[:, 1, :], w1_d[128:256, :])
        w2o_sb = const_pool.tile([P, 2], _BF16, tag="w2")
        nc.sync.dma_start(w2o_sb[:], w2_d[:])
        w2_sb = w2o_sb[:, 0:1]
        ones_sb = w2o_sb[:, 1:2]
        b1_sb = const_pool.tile([P, 1], _F32, tag="b1")
        nc.sync.dma_start(b1_sb[:], b1_d[:])
        bt_sb = const_pool.tile([P, nt + SEGS_PER_CORE], _I32, tag="bt")
        nc.sync.dma_start(bt_sb[:], bt_d[:])
        iota_sb = bt_sb[:, nt:nt + SEGS_PER_CORE]

        acc_ps = psum_acc.tile([SEGS_PER_CORE, D], _F32, tag="acc")
        den_ps = psum_acc.tile([SEGS_PER_CORE, 1], _F32, tag="den")

        rep_ctx = tc.For_i(0, reps, 1) if reps > 1 else None
        if rep_ctx is not None:
            rep_ctx.__enter__()
        for c in range(n_chunks):
            # 1) load + cast x chunk (node-partitioned)
            x_bf = xbf_pool.tile([P, CHUNK_T, D], _BF16, tag="xbf")
            nc.gpsimd.dma_start(x_bf[:], x_ap[c])

            # 2) bounce bf16 chunk to DRAM (ACT hwdge ring), then two big
            #    xbar transpose loads (SP hwdge ring) -> channel-partitioned
            xd = xd_pool.tile([CHUNK_N, D], _BF16, tag="xd")
            nc.scalar.dma_start(
                xd[:].rearrange("(t p) ch -> p t ch", p=P), x_bf[:])
            xt_lo = xt_pool.tile([P, CHUNK_N], _BF16, tag="xtlo")
            xt_hi = xt_pool.tile([P, CHUNK_N], _BF16, tag="xthi")
            nc.sync.dma_start_transpose(xt_lo[:], xd[:, 0:128])
            nc.sync.dma_start_transpose(xt_hi[:], xd[:, 128:256])

            # 3) h = tanh(x @ W1 + b1), hidden-partitioned, bf16
            h_bf = h_pool.tile([P, CHUNK_N], _BF16, tag="h")
            for s in range(CHUNK_N // 512):
                ph = psum_h.tile([P, 512], _F32, tag="ph")
                sl = slice(s * 512, (s + 1) * 512)
                nc.tensor.matmul(ph[:], w1_sb[:, 0, :], xt_lo[:, sl],
                                 start=True, stop=False)
                nc.tensor.matmul(ph[:], w1_sb[:, 1, :], xt_hi[:, sl],
                                 start=False, stop=True)
                nc.scalar.activation(h_bf[:, sl], ph[:],
                                     mybir.ActivationFunctionType.Tanh,
                                     bias=b1_sb[:])

            # 4) per-tile score columns: s_col[p, t] = h_tile.T @ W2
            ps_s = psum_s.tile([P, CHUNK_T], _F32, tag="ps_s")
            for t in range(CHUNK_T):
                nc.tensor.matmul(ps_s[:, t:t + 1],
                                 h_bf[:, t * TILE_N:(t + 1) * TILE_N],
                                 w2_sb, start=True, stop=True)

            # 5) e = exp(s + b2)  (node-partitioned, fp32)
            e_col = ecol_pool.tile([P, CHUNK_T], _F32, tag="ecol")
            nc.scalar.activation(e_col[:], ps_s[:],
                                 mybir.ActivationFunctionType.Exp,
                                 bias=float(b2_val))

            # 6) we[p, t, g] = (batch_t == g) * e   (bf16)
            cmp = we_pool.tile([P, CHUNK_T, SEGS_PER_CORE], _BF16, tag="cmp")
            bt_c = bt_sb[:, c * CHUNK_T:(c + 1) * CHUNK_T]
            nc.vector.tensor_tensor(
                cmp[:],
                bt_c.unsqueeze(2).broadcast_to([P, CHUNK_T, SEGS_PER_CORE]),
                iota_sb.unsqueeze(1).broadcast_to([P, CHUNK_T, SEGS_PER_CORE]),
                mybir.AluOpType.is_equal)
            we = we_pool.tile([P, CHUNK_T, SEGS_PER_CORE], _BF16, tag="we")
            nc.vector.memset(we[:], 0.0)
            nc.vector.tensor_tensor(
                we[:], cmp[:],
                e_col[:].unsqueeze(2).broadcast_to([P, CHUNK_T, SEGS_PER_CORE]),
                mybir.AluOpType.mult)

            # 7) weighted segment sums + denominators, accumulated in PSUM
            first = c == 0
            last = c == n_chunks - 1
            for t in range(CHUNK_T):
                nc.tensor.matmul(acc_ps[:], we[:, t, :], x_bf[:, t, :],
                                 start=(first and t == 0),
                                 stop=(last and t == CHUNK_T - 1),
                                 skip_group_check=True)
                nc.tensor.matmul(den_ps[:], we[:, t, :], ones_sb,
                                 start=(first and t == 0),
                                 stop=(last and t == CHUNK_T - 1),
                                 skip_group_check=True)

        if rep_ctx is not None:
            rep_ctx.__exit__(None, None, None)

        # ---- epilogue: out = acc / den ----
        den_sb = fin_pool.tile([SEGS_PER_CORE, 1], _F32, tag="den_sb")
        nc.vector.tensor_scalar_add(den_sb[:], den_ps[:], 1e-30)
        rec_sb = fin_pool.tile([SEGS_PER_CORE, 1], _F32, tag="rec_sb")
        nc.vector.reciprocal(rec_sb[:], den_sb[:])
        out_sb = fin_pool.tile([SEGS_PER_CORE, D], _F32, tag="out_sb")
        nc.vector.tensor_scalar_mul(out_sb[:], acc_ps[:], rec_sb[:])
        nc.sync.dma_start(out_d[:], out_sb[:])

    return nc


def _prepare_inputs(x, W1, b1, W2, b2, batch):
    n = x.shape[0]
    batch = np.asarray(batch).astype(np.int64)
    # core k owns segments [64k, 64(k+1)); sorted batch -> contiguous ranges
    bounds = np.searchsorted(batch, np.arange(0, NUM_GRAPHS + 1, SEGS_PER_CORE))
    counts = np.diff(bounds)
    nmax = int(np.max(counts))
    n_chunks = max(1, (nmax + CHUNK_N - 1) // CHUNK_N)
    nmax_pad = n_chunks * CHUNK_N

    w1_bf = np.asarray(W1, np.float32).astype(ml_dtypes.bfloat16)
    w2_bf = np.concatenate([np.asarray(W2, np.float32).reshape(H, 1),
                            np.ones((H, 1), np.float32)], 1).astype(ml_dtypes.bfloat16)
    b1_col = np.asarray(b1, np.float32).reshape(H, 1)

    in_maps = []
    for k in range(N_CORES):
        lo, hi = int(bounds[k]), int(bounds[k + 1])
        cnt = hi - lo
        x_pad = np.zeros((nmax_pad, D), np.float32)
        x_pad[:cnt] = x[lo:hi]
        bt = np.full((nmax_pad,), -1, np.int32)
        bt[:cnt] = batch[lo:hi] - k * SEGS_PER_CORE
        bt_t = bt.reshape(nmax_pad // P, P).T  # (128, nt)
        iota_cols = np.tile(np.arange(SEGS_PER_CORE, dtype=np.int32), (P, 1))
        bt_t = np.concatenate([bt_t, iota_cols], axis=1).copy()
        in_maps.append({
            "x": x_pad,
            "batch_t": bt_t,
            "w1": w1_bf,
            "w2": w2_bf,
            "b1": b1_col,
        })
    return in_maps, n_chunks


def run(x, W1, b1, W2, b2, batch, trace=False, trace_kwargs=None):
    in_maps, n_chunks = _prepare_inputs(x, W1, b1, W2, b2, batch)
    nc = _build_program(n_chunks, float(np.asarray(b2).reshape(-1)[0]))
    nc.finalize()
    res = run_bass_kernel_spmd(nc, in_maps, list(range(N_CORES)),
                               trace=trace, **(trace_kwargs or {}))
    out = np.concatenate([np.asarray(res.results[k]["out_g"], np.float32)
                          for k in range(N_CORES)], axis=0)
    return out, res


def kernel(x, W1, b1, W2, b2, batch):
    out, _ = run(x, W1, b1, W2, b2, batch)
    return out
